# revision 1
# baseline (speedup 1.0000x reference)
"""Trainium2 Bass kernel for nn_BackwardCompatibleLoss.

Strategy (data-parallel over batch rows, 8 NeuronCores):

Host side (data movement only):
  - Rows are sorted by target label (the loss is permutation-invariant over
    batch rows).  After sorting, every same-label group is a contiguous row
    range, so for each core's 512-row shard all same-label partners lie in a
    fixed-size "window" of rows around the shard.
  - Each core receives its window of raw feat/feat_old rows, the window/local
    targets (as f32), a per-core 0/1 weight vector (0 on its window rows) and
    an identity matrix constant.

Device side (all O(B*D) and O(B^2) math):
  - Each core L2-normalizes its window rows (bn_stats -> sqrt -> reciprocal),
    casts to bf16 and transposes to [D, rows] layout via DMA-transpose.
  - Two AllGathers (fo first, then fn; the second hides behind the n2o sweep)
    of each core's transposed local 512-row block give every core the full
    [D, 4096] normalized feature matrices.
  - Main compute per 128-row j-tile (transposed orientation S^T[j, i]):
    PSUM = matmul over 4 d-blocks; E = exp(100*S - 35) on ScalarE (the -35
    shift keeps every exponent in fp32/bf16 normal range: unmasked cosines
    are <~0.3, and the n2n diagonal's exp(65) stays finite and gets zero
    weight); Z[1, 512] accumulates in PSUM via a weights-vector matmul
    (partition-axis reduction on the TensorEngine).
  - Window j-tiles take the same-label additive mask (-1e9), built on-device
    from target equality, before the exp and weight 1; global-sweep tiles are
    weighted by w (0 on window rows) so each j contributes exactly once.
  - The positive logit is the diagonal of the window n2o product (identity
    mask + ones-matmul).  loss_i = ln(Z_i) + 35 - 100*pos_i, summed to a
    per-core partial.

  Top-k(1024) in the reference is replaced by the full masked logsumexp: with
  temperature 0.01 the excluded tail contributes ~2e-6 relative error.

Host sums the 8 partial outputs -> mean.
"""

import sys

if "/opt/trn_rl_repo" not in sys.path:
    sys.path.insert(0, "/opt/trn_rl_repo")

import math
from contextlib import ExitStack

import numpy as np

import concourse.bacc as bacc
import concourse.bass as bass
import concourse.tile as tile
from concourse import mybir
from concourse.bass_utils import run_bass_kernel_spmd

F32 = mybir.dt.float32
BF16 = mybir.dt.bfloat16
NP_BF16 = mybir.dt.np(BF16)
AF = mybir.ActivationFunctionType
ALU = mybir.AluOpType

B, D = 4096, 512
NCORES = 8
BL = B // NCORES          # 512 local rows per core
NDB = D // 128            # 4 contraction blocks
NGT = B // 128            # 32 global j-tiles
TEMP = 0.01
SCALE = 1.0 / TEMP        # 100
EBIAS = -35.0             # exp(100*S - 35): keeps all exponents in fp32 range
NEG = -1.0e9

_cache = {}


def _build(wtiles: int):
    """Build + compile the SPMD program. wtiles = window size in 128-row tiles."""
    WIN = wtiles * 128
    LPAD = ((wtiles - 4) // 2) * 128          # rows of left padding in window
    LT = LPAD // 128

    nc = bacc.Bacc("TRN2", target_bir_lowering=False, debug=False,
                   num_devices=NCORES)

    xw = nc.dram_tensor("xw", [WIN, D], F32, kind="ExternalInput")
    yw = nc.dram_tensor("yw", [WIN, D], F32, kind="ExternalInput")
    tw = nc.dram_tensor("tw", [WIN], F32, kind="ExternalInput")
    tl = nc.dram_tensor("tl", [BL], F32, kind="ExternalInput")
    wv = nc.dram_tensor("wv", [B], BF16, kind="ExternalInput")
    idm = nc.dram_tensor("idm", [128, 128], F32, kind="ExternalInput")
    outp = nc.dram_tensor("outp", [1, 1], F32, kind="ExternalOutput")

    natf = nc.dram_tensor("natf", [WIN, D], BF16)
    nato = nc.dram_tensor("nato", [WIN, D], BF16)
    ccin = nc.dram_tensor("ccin", [2, D, BL], BF16)
    ccout = nc.dram_tensor("ccout", [NCORES, 2, D, BL], BF16,
                           addr_space="Shared")

    with ExitStack() as ctx:
        tc = ctx.enter_context(tile.TileContext(nc))
        singles = ctx.enter_context(tc.tile_pool(name="singles", bufs=1))
        work = ctx.enter_context(tc.tile_pool(name="work", bufs=3))
        epool = ctx.enter_context(tc.tile_pool(name="epool", bufs=4))
        psS = ctx.enter_context(tc.tile_pool(name="psS", bufs=4, space="PSUM"))
        psA = ctx.enter_context(tc.tile_pool(name="psA", bufs=1, space="PSUM"))

        # persistent SBUF tensors
        fnT = singles.tile([128, NDB, WIN], BF16, tag="fnT")
        foT = singles.tile([128, NDB, WIN], BF16, tag="foT")
        gT = singles.tile([128, 2, NDB, B], BF16, tag="gT")
        tlb = singles.tile([128, BL], F32, tag="tlb")
        twc = singles.tile([128, wtiles], F32, tag="twc")
        wcol = singles.tile([128, NGT], BF16, tag="wcol")
        identS = singles.tile([128, 128], F32, tag="identS")
        ones_bf = singles.tile([128, 1], BF16, tag="ones_bf")
        ones_f = singles.tile([128, 1], F32, tag="ones_f")
        ebias = singles.tile([128, 1], F32, tag="ebias")
        psZ = psA.tile([1, BL], F32, tag="psZ")
        psP = psA.tile([1, BL], F32, tag="psP")

        nc.vector.memset(ones_bf, 1.0)
        nc.vector.memset(ebias, EBIAS)
        nc.vector.memset(ones_f, 1.0)
        nc.sync.dma_start(out=identS, in_=idm[:, :])
        tl_ap = tl.ap()
        nc.sync.dma_start(
            out=tlb,
            in_=bass.AP(tensor=tl_ap.tensor, offset=tl_ap.offset,
                        ap=[[0, 128]] + list(tl_ap.ap)),
        )
        nc.sync.dma_start(out=twc, in_=tw.ap().rearrange("(s p) -> p s", p=128))
        nc.sync.dma_start(out=wcol, in_=wv.ap().rearrange("(g p) -> p g", p=128))

        def norm_block(src, nat, b):
            xb = work.tile([128, D], F32, tag="xb")
            nc.sync.dma_start(out=xb, in_=src[b * 128:(b + 1) * 128, :])
            st = work.tile([128, 6], F32, tag="st")
            nc.vector.bn_stats(out=st, in_=xb)
            mv = work.tile([128, 2], F32, tag="mv")
            nc.vector.bn_aggr(out=mv, in_=st)
            m2 = work.tile([128, 1], F32, tag="m2")
            nc.vector.tensor_mul(out=m2, in0=mv[:, 0:1], in1=mv[:, 0:1])
            ex2 = work.tile([128, 1], F32, tag="ex2")
            nc.vector.tensor_add(out=ex2, in0=m2, in1=mv[:, 1:2])
            nrm = work.tile([128, 1], F32, tag="nrm")
            nc.scalar.activation(out=nrm, in_=ex2, func=AF.Sqrt,
                                 scale=float(D))
            rs = work.tile([128, 1], F32, tag="rs")
            nc.vector.reciprocal(out=rs, in_=nrm)
            nb = work.tile([128, D], BF16, tag="nb")
            nc.vector.tensor_scalar_mul(out=nb, in0=xb, scalar1=rs)
            nc.sync.dma_start(out=nat[b * 128:(b + 1) * 128, :], in_=nb)

        # ---- Phase A: normalize window rows ----
        for src, nat in ((xw, natf), (yw, nato)):
            for b in range(wtiles):
                norm_block(src, nat, b)

        # ---- Phase B: transpose-load windows (all before any collective),
        #      then the two AllGathers: fo first, fn second ----
        for nat, dstT in ((natf, fnT), (nato, foT)):
            for db in range(NDB):
                nc.sync.dma_start_transpose(
                    out=dstT[:, db, :],
                    in_=nat[:, db * 128:(db + 1) * 128])
        nc.sync.dma_start(out=ccin[0, :, :].rearrange("(a p) j -> p a j", p=128),
                          in_=fnT[:, :, LPAD:LPAD + BL])
        nc.sync.dma_start(out=ccin[1, :, :].rearrange("(a p) j -> p a j", p=128),
                          in_=foT[:, :, LPAD:LPAD + BL])
        nc.gpsimd.collective_compute(
            "AllGather",
            ALU.bypass,
            replica_groups=[list(range(NCORES))],
            ins=[ccin.ap().opt()],
            outs=[ccout.ap().opt()],
        )

        rhs_loc = fnT[:, :, LPAD:LPAD + BL]   # [128, NDB, 512] local fn cols

        # ---- Phase C: window pass (same-label masking + positive logits) ----
        first_z = True
        for s in range(wtiles):
            eqm = work.tile([128, BL], F32, tag="eqm")
            nc.vector.tensor_scalar(
                out=eqm, in0=tlb, scalar1=twc[:, s:s + 1], scalar2=NEG,
                op0=ALU.is_equal, op1=ALU.mult)
            for t, lhsrc in ((0, foT), (1, fnT)):
                ps = psS.tile([128, BL], F32, tag="ps")
                for db in range(NDB):
                    nc.tensor.matmul(
                        ps, lhsrc[:, db, s * 128:(s + 1) * 128],
                        rhs_loc[:, db, :],
                        start=(db == 0), stop=(db == NDB - 1),
                        skip_group_check=True)
                if t == 0 and LT <= s < LT + 4:
                    k = s - LT
                    tmp = work.tile([128, 128], F32, tag="diag")
                    nc.vector.tensor_mul(out=tmp,
                                         in0=ps[:, k * 128:(k + 1) * 128],
                                         in1=identS)
                    nc.tensor.matmul(psP[0:1, k * 128:(k + 1) * 128],
                                     ones_f, tmp, start=True, stop=True,
                                     skip_group_check=True)
                nc.vector.tensor_add(out=ps, in0=ps, in1=eqm)
                E = epool.tile([128, BL], BF16, tag="E")
                nc.scalar.activation(out=E, in_=ps, func=AF.Exp,
                                     bias=ebias, scale=SCALE)
                nc.tensor.matmul(psZ[0:1, :], ones_bf, E,
                                 start=first_z, stop=False,
                                 skip_group_check=True)
                first_z = False

        # ---- Phase D: global sweep over gathered features ----
        for r in range(NCORES):
            for t in range(2):
                for db in range(NDB):
                    nc.sync.dma_start(
                        out=gT[:, t, db, r * BL:(r + 1) * BL],
                        in_=ccout[r, t, db * 128:(db + 1) * 128, :])
        for r in range(NCORES):
            for t, tg in ((0, 1), (1, 0)):
                for j4 in range(4):
                    g = r * 4 + j4
                    ps = psS.tile([128, BL], F32, tag="ps")
                    for db in range(NDB):
                        nc.tensor.matmul(
                            ps, gT[:, tg, db, g * 128:(g + 1) * 128],
                            rhs_loc[:, db, :],
                            start=(db == 0), stop=(db == NDB - 1),
                            skip_group_check=True)
                    E = epool.tile([128, BL], BF16, tag="E")
                    nc.scalar.activation(out=E, in_=ps, func=AF.Exp,
                                         bias=ebias, scale=SCALE)
                    last = (t == 1 and r == NCORES - 1 and j4 == 3)
                    nc.tensor.matmul(psZ[0:1, :], wcol[:, g:g + 1], E,
                                     start=False, stop=last,
                                     skip_group_check=True)

        # ---- Phase E: loss tail ----
        lnz = singles.tile([1, BL], F32, tag="lnz")
        nc.scalar.activation(out=lnz, in_=psZ[0:1, :], func=AF.Ln,
                             scale=float(math.exp(-EBIAS)))
        pos100 = singles.tile([1, BL], F32, tag="pos100")
        nc.scalar.activation(out=pos100, in_=psP[0:1, :], func=AF.Copy,
                             scale=SCALE)
        lv = singles.tile([1, BL], F32, tag="lv")
        nc.vector.tensor_sub(out=lv, in0=lnz, in1=pos100)
        part = singles.tile([1, 1], F32, tag="part")
        nc.vector.reduce_sum(out=part, in_=lv, axis=mybir.AxisListType.X)
        nc.sync.dma_start(out=outp[0:1, 0:1], in_=part)

    nc.compile()
    return nc


def kernel(feat: np.ndarray, feat_old: np.ndarray,
           targets: np.ndarray) -> np.ndarray:
    feat = np.asarray(feat, dtype=np.float32)
    feat_old = np.asarray(feat_old, dtype=np.float32)
    targets_np = np.asarray(targets)

    # sort rows by label: same-label groups become contiguous
    order = np.argsort(targets_np, kind="stable")
    fs = np.ascontiguousarray(feat[order])
    fo = np.ascontiguousarray(feat_old[order])
    ts = targets_np[order].astype(np.float32)

    # window padding must cover the largest same-label group
    _, counts = np.unique(targets_np, return_counts=True)
    maxc = int(counts.max()) if counts.size else 1
    lpad_tiles = max(1, -(-(maxc - 1) // 128))
    wtiles = 4 + 2 * lpad_tiles
    LPAD = lpad_tiles * 128
    WIN = wtiles * 128

    key = wtiles
    if key not in _cache:
        _cache[key] = _build(wtiles)
    nc = _cache[key]

    idm = np.eye(128, dtype=np.float32)
    in_maps = []
    for c in range(NCORES):
        idx = (np.arange(c * BL - LPAD, c * BL - LPAD + WIN)) % B
        wvec = np.ones(B, dtype=NP_BF16)
        wvec[idx] = 0
        in_maps.append({
            "xw": np.ascontiguousarray(fs[idx]),
            "yw": np.ascontiguousarray(fo[idx]),
            "tw": np.ascontiguousarray(ts[idx]),
            "tl": np.ascontiguousarray(ts[c * BL:(c + 1) * BL]),
            "wv": wvec,
            "idm": idm,
        })

    res = run_bass_kernel_spmd(nc, in_maps, core_ids=list(range(NCORES)))
    total = sum(float(res.results[c]["outp"][0, 0]) for c in range(NCORES))
    return np.asarray(np.float32(total / B))


if __name__ == "__main__":
    rng = np.random.default_rng(0)
    f = rng.standard_normal((B, D)).astype(np.float32)
    g = rng.standard_normal((B, D)).astype(np.float32)
    t = rng.integers(0, 1000, size=B).astype(np.int64)
    print("loss:", kernel(f, g, t))



# revision 2
# speedup vs baseline: 1.1544x; 1.1544x over previous
"""Trainium2 Bass kernel for nn_BackwardCompatibleLoss.

Strategy (data-parallel over batch rows, 8 NeuronCores):

Host side (data movement only):
  - Rows are sorted by target label (the loss is permutation-invariant over
    batch rows).  After sorting, every same-label group is a contiguous row
    range, so for each core's 512-row shard all same-label partners lie in a
    fixed-size "window" of rows around the shard.
  - Each core receives its window of raw feat/feat_old rows, the window/local
    targets (as f32), a per-core 0/1 weight vector (0 on its window rows) and
    an identity matrix constant.

Device side (all O(B*D) and O(B^2) math):
  - Each core L2-normalizes its window rows (bn_stats -> sqrt -> reciprocal),
    scales by 64, casts to bf16, DMA-transposes to [D, rows] layout, then
    casts to fp8 e4m3 (values <= 16, well inside e4m3 range).
  - The local 512-row slice of feat_old is processed FIRST and AllGathered
    immediately; feat follows in a second AllGather.  The new-to-old sweep
    only needs the first gather, so it overlaps the second; the window pass
    (local data only) overlaps the first.
  - Main compute per 128-row j-tile (transposed orientation S^T[j, i]):
    PSUM q = 4096*S via two fp8 DoubleRow matmuls (2 contraction slabs each);
    E = exp(q/40.96 - 35) on ScalarE (the -35 shift keeps every exponent in
    range); Z[1, 512] accumulates in PSUM via a weights-vector matmul
    (partition-axis reduction on the TensorEngine).
  - Window j-tiles take the same-label additive mask (-1e9), built on-device
    from target equality, before the exp and weight 1; global-sweep tiles are
    weighted by w (0 on window rows) so each j contributes exactly once.
  - The positive logit is the diagonal of the window n2o product (identity
    mask + ones-matmul).  loss_i = ln(Z_i) + 35 - q_pos_i/40.96, summed to a
    per-core partial.

  Top-k(1024) in the reference is replaced by the full masked logsumexp: with
  temperature 0.01 the excluded tail contributes ~2e-6 relative error.  The
  fp8 feature quantization adds ~1e-3 relative error on the loss.

Host sums the 8 partial outputs -> mean.
"""

import sys

if "/opt/trn_rl_repo" not in sys.path:
    sys.path.insert(0, "/opt/trn_rl_repo")

import math
from contextlib import ExitStack

import numpy as np

import concourse.bacc as bacc
import concourse.bass as bass
import concourse.tile as tile
from concourse import mybir
from concourse.bass_utils import run_bass_kernel_spmd

F32 = mybir.dt.float32
BF16 = mybir.dt.bfloat16
FP8 = mybir.dt.float8e4
NP_BF16 = mybir.dt.np(BF16)
AF = mybir.ActivationFunctionType
ALU = mybir.AluOpType
DR = mybir.MatmulPerfMode.DoubleRow

B, D = 4096, 512
NCORES = 8
BL = B // NCORES          # 512 local rows per core
NDB = D // 128            # 4 contraction blocks
NGT = B // 128            # 32 global j-tiles
TEMP = 0.01
QS = 64.0                 # fp8 feature scale: q = 4096 * S
SCALE_Q = (1.0 / TEMP) / (QS * QS)   # exp scale on raw psum
EBIAS = -35.0             # exp(q*SCALE_Q - 35): keeps exponents in f32 range
NEG = -1.0e9

_cache = {}


def _build(wtiles: int, use_fp8: bool = True):
    """Build + compile the SPMD program. wtiles = window size in 128-row tiles."""
    WIN = wtiles * 128
    LPAD = ((wtiles - 4) // 2) * 128          # rows of left padding in window
    LT = LPAD // 128

    QDT = FP8 if use_fp8 else BF16
    qscale = QS if use_fp8 else 1.0
    sc_q = SCALE_Q if use_fp8 else 1.0 / TEMP

    nc = bacc.Bacc("TRN2", target_bir_lowering=False, debug=False,
                   num_devices=NCORES)

    xw = nc.dram_tensor("xw", [WIN, D], F32, kind="ExternalInput")
    yw = nc.dram_tensor("yw", [WIN, D], F32, kind="ExternalInput")
    tw = nc.dram_tensor("tw", [WIN], F32, kind="ExternalInput")
    tl = nc.dram_tensor("tl", [BL], F32, kind="ExternalInput")
    wv = nc.dram_tensor("wv", [B], BF16, kind="ExternalInput")
    idm = nc.dram_tensor("idm", [128, 128], F32, kind="ExternalInput")
    outp = nc.dram_tensor("outp", [1, 1], F32, kind="ExternalOutput")

    natf = nc.dram_tensor("natf", [WIN, D], BF16)
    nato = nc.dram_tensor("nato", [WIN, D], BF16)
    ccin_o = nc.dram_tensor("ccin_o", [D, BL], QDT)
    ccin_f = nc.dram_tensor("ccin_f", [D, BL], QDT)
    ccout_o = nc.dram_tensor("ccout_o", [NCORES, D, BL], QDT,
                             addr_space="Shared")
    ccout_f = nc.dram_tensor("ccout_f", [NCORES, D, BL], QDT,
                             addr_space="Shared")

    with ExitStack() as ctx:
        tc = ctx.enter_context(tile.TileContext(nc))
        singles = ctx.enter_context(tc.tile_pool(name="singles", bufs=1))
        work = ctx.enter_context(tc.tile_pool(name="work", bufs=3))
        epool = ctx.enter_context(tc.tile_pool(name="epool", bufs=4))
        psS = ctx.enter_context(tc.tile_pool(name="psS", bufs=5, space="PSUM"))
        psA = ctx.enter_context(tc.tile_pool(name="psA", bufs=1, space="PSUM"))

        # persistent SBUF tensors
        fnT = singles.tile([128, NDB, WIN], BF16, tag="fnT")
        foT = singles.tile([128, NDB, WIN], BF16, tag="foT")
        fnTq = singles.tile([128, NDB, WIN], QDT, tag="fnTq")
        foTq = singles.tile([128, NDB, WIN], QDT, tag="foTq")
        gTo = singles.tile([128, NDB, B], QDT, tag="gTo")
        gTn = singles.tile([128, NDB, B], QDT, tag="gTn")
        tlb = singles.tile([128, BL], F32, tag="tlb")
        twc = singles.tile([128, wtiles], F32, tag="twc")
        wcol = singles.tile([128, NGT], BF16, tag="wcol")
        identS = singles.tile([128, 128], F32, tag="identS")
        ones_bf = singles.tile([128, 1], BF16, tag="ones_bf")
        ones_f = singles.tile([128, 1], F32, tag="ones_f")
        ebias = singles.tile([128, 1], F32, tag="ebias")
        psZ = psA.tile([1, BL], F32, tag="psZ")
        psP = psA.tile([1, BL], F32, tag="psP")

        nc.vector.memset(ones_bf, 1.0)
        nc.vector.memset(ebias, EBIAS)
        nc.vector.memset(ones_f, 1.0)
        nc.sync.dma_start(out=identS, in_=idm[:, :])
        tl_ap = tl.ap()
        nc.sync.dma_start(
            out=tlb,
            in_=bass.AP(tensor=tl_ap.tensor, offset=tl_ap.offset,
                        ap=[[0, 128]] + list(tl_ap.ap)),
        )
        nc.sync.dma_start(out=twc, in_=tw.ap().rearrange("(s p) -> p s", p=128))
        nc.sync.dma_start(out=wcol, in_=wv.ap().rearrange("(g p) -> p g", p=128))

        def norm_block(src, nat, b):
            """Normalize rows [b*128, (b+1)*128) of src, scaled by qscale."""
            xb = work.tile([128, D], F32, tag="xb")
            nc.sync.dma_start(out=xb, in_=src[b * 128:(b + 1) * 128, :])
            st = work.tile([128, 6], F32, tag="st")
            nc.vector.bn_stats(out=st, in_=xb)
            mv = work.tile([128, 2], F32, tag="mv")
            nc.vector.bn_aggr(out=mv, in_=st)
            m2 = work.tile([128, 1], F32, tag="m2")
            nc.vector.tensor_mul(out=m2, in0=mv[:, 0:1], in1=mv[:, 0:1])
            ex2 = work.tile([128, 1], F32, tag="ex2")
            nc.vector.tensor_add(out=ex2, in0=m2, in1=mv[:, 1:2])
            nrm = work.tile([128, 1], F32, tag="nrm")
            # nrm = ||x|| / qscale  ->  rs = qscale / ||x||
            nc.scalar.activation(out=nrm, in_=ex2, func=AF.Sqrt,
                                 scale=float(D) / (qscale * qscale))
            rs = work.tile([128, 1], F32, tag="rs")
            nc.vector.reciprocal(out=rs, in_=nrm)
            nb = work.tile([128, D], BF16, tag="nb")
            nc.vector.tensor_scalar_mul(out=nb, in0=xb, scalar1=rs)
            nc.sync.dma_start(out=nat[b * 128:(b + 1) * 128, :], in_=nb)

        def transpose_rows(nat, dstT, r0, r1):
            for db in range(NDB):
                nc.sync.dma_start_transpose(
                    out=dstT[:, db, r0:r1],
                    in_=nat[r0:r1, db * 128:(db + 1) * 128])

        def cast_q(srcT, dstTq, r0, r1):
            if use_fp8:
                nc.scalar.activation(out=dstTq[:, :, r0:r1],
                                     in_=srcT[:, :, r0:r1], func=AF.Copy)

        # ---- Phase A+B, local rows first, AllGather(fo) then AllGather(fn).
        # fo first: the global n2o sweep only needs ccout_o.
        for src, nat, natT, natTq, ccin, ccout in (
                (yw, nato, foT, foTq, ccin_o, ccout_o),
                (xw, natf, fnT, fnTq, ccin_f, ccout_f)):
            for s in range(LT, LT + 4):
                norm_block(src, nat, s)
            transpose_rows(nat, natT, LPAD, LPAD + BL)
            cast_q(natT, natTq, LPAD, LPAD + BL)
            natQ = natTq if use_fp8 else natT
            nc.sync.dma_start(
                out=ccin.ap().rearrange("(a p) j -> p a j", p=128),
                in_=natQ[:, :, LPAD:LPAD + BL])
            nc.gpsimd.collective_compute(
                "AllGather",
                ALU.bypass,
                replica_groups=[list(range(NCORES))],
                ins=[ccin.ap().opt()],
                outs=[ccout.ap().opt()],
            )

        # ---- pad rows of the window (needed only for the window pass) ----
        pads = [s for s in range(wtiles) if not (LT <= s < LT + 4)]
        for src, nat, natT, natTq in ((yw, nato, foT, foTq),
                                      (xw, natf, fnT, fnTq)):
            for s in pads:
                norm_block(src, nat, s)
            for s in pads:
                transpose_rows(nat, natT, s * 128, (s + 1) * 128)
                cast_q(natT, natTq, s * 128, (s + 1) * 128)

        loc_q = (fnTq if use_fp8 else fnT)
        rhs_loc = loc_q[:, :, LPAD:LPAD + BL]   # [128, NDB, 512] local fn cols

        def mm_group(ps, lhs_src, j0):
            """PSUM[j 128, i 512] = sum_d lhs_src[d, j0:j0+128] * local fn."""
            if use_fp8:
                for p in range(2):
                    nc.tensor.matmul(
                        ps, lhs_src[:, 2 * p:2 * p + 2, j0:j0 + 128],
                        rhs_loc[:, 2 * p:2 * p + 2, :],
                        start=(p == 0), stop=(p == 1),
                        perf_mode=DR, skip_group_check=True)
            else:
                for db in range(NDB):
                    nc.tensor.matmul(
                        ps, lhs_src[:, db, j0:j0 + 128],
                        rhs_loc[:, db, :],
                        start=(db == 0), stop=(db == NDB - 1),
                        skip_group_check=True)

        # ---- Phase C: window pass (same-label masking + positive logits) ----
        first_z = True
        for s in range(wtiles):
            eqm = work.tile([128, BL], F32, tag="eqm")
            nc.vector.tensor_scalar(
                out=eqm, in0=tlb, scalar1=twc[:, s:s + 1], scalar2=NEG,
                op0=ALU.is_equal, op1=ALU.mult)
            for t, lhsrc in ((0, foTq if use_fp8 else foT),
                             (1, loc_q)):
                ps = psS.tile([128, BL], F32, tag="ps")
                mm_group(ps, lhsrc, s * 128)
                if t == 0 and LT <= s < LT + 4:
                    k = s - LT
                    tmp = work.tile([128, 128], F32, tag="diag")
                    nc.vector.tensor_mul(out=tmp,
                                         in0=ps[:, k * 128:(k + 1) * 128],
                                         in1=identS)
                    nc.tensor.matmul(psP[0:1, k * 128:(k + 1) * 128],
                                     ones_f, tmp, start=True, stop=True,
                                     skip_group_check=True)
                nc.vector.tensor_add(out=ps, in0=ps, in1=eqm)
                E = epool.tile([128, BL], BF16, tag="E")
                nc.scalar.activation(out=E, in_=ps, func=AF.Exp,
                                     bias=ebias, scale=sc_q)
                nc.tensor.matmul(psZ[0:1, :], ones_bf, E,
                                 start=first_z, stop=False,
                                 skip_group_check=True)
                first_z = False

        # ---- Phase D: global sweep; n2o (gTo) first so it only waits on the
        # first AllGather, n2n (gTn) second ----
        for t, ccout, gT in ((0, ccout_o, gTo), (1, ccout_f, gTn)):
            for r in range(NCORES):
                nc.sync.dma_start(
                    out=gT[:, :, r * BL:(r + 1) * BL],
                    in_=ccout[r].rearrange("(a p) j -> p a j", p=128))
            for g in range(NGT):
                ps = psS.tile([128, BL], F32, tag="ps")
                mm_group(ps, gT, g * 128)
                E = epool.tile([128, BL], BF16, tag="E")
                nc.scalar.activation(out=E, in_=ps, func=AF.Exp,
                                     bias=ebias, scale=sc_q)
                last = (t == 1 and g == NGT - 1)
                nc.tensor.matmul(psZ[0:1, :], wcol[:, g:g + 1], E,
                                 start=False, stop=last,
                                 skip_group_check=True)

        # ---- Phase E: loss tail ----
        lnz = singles.tile([1, BL], F32, tag="lnz")
        nc.scalar.activation(out=lnz, in_=psZ[0:1, :], func=AF.Ln,
                             scale=float(math.exp(-EBIAS)))
        pos100 = singles.tile([1, BL], F32, tag="pos100")
        nc.scalar.activation(out=pos100, in_=psP[0:1, :], func=AF.Copy,
                             scale=sc_q)
        lv = singles.tile([1, BL], F32, tag="lv")
        nc.vector.tensor_sub(out=lv, in0=lnz, in1=pos100)
        part = singles.tile([1, 1], F32, tag="part")
        nc.vector.reduce_sum(out=part, in_=lv, axis=mybir.AxisListType.X)
        nc.sync.dma_start(out=outp[0:1, 0:1], in_=part)

    nc.compile()
    return nc


def kernel(feat: np.ndarray, feat_old: np.ndarray,
           targets: np.ndarray) -> np.ndarray:
    feat = np.asarray(feat, dtype=np.float32)
    feat_old = np.asarray(feat_old, dtype=np.float32)
    targets_np = np.asarray(targets)

    # sort rows by label: same-label groups become contiguous
    order = np.argsort(targets_np, kind="stable")
    fs = np.ascontiguousarray(feat[order])
    fo = np.ascontiguousarray(feat_old[order])
    ts = targets_np[order].astype(np.float32)

    # window padding must cover the largest same-label group
    _, counts = np.unique(targets_np, return_counts=True)
    maxc = int(counts.max()) if counts.size else 1
    lpad_tiles = max(1, -(-(maxc - 1) // 128))
    wtiles = 4 + 2 * lpad_tiles
    LPAD = lpad_tiles * 128
    WIN = wtiles * 128

    key = wtiles
    if key not in _cache:
        _cache[key] = _build(wtiles)
    nc = _cache[key]

    idm = np.eye(128, dtype=np.float32)
    in_maps = []
    for c in range(NCORES):
        idx = (np.arange(c * BL - LPAD, c * BL - LPAD + WIN)) % B
        wvec = np.ones(B, dtype=NP_BF16)
        wvec[idx] = 0
        in_maps.append({
            "xw": np.ascontiguousarray(fs[idx]),
            "yw": np.ascontiguousarray(fo[idx]),
            "tw": np.ascontiguousarray(ts[idx]),
            "tl": np.ascontiguousarray(ts[c * BL:(c + 1) * BL]),
            "wv": wvec,
            "idm": idm,
        })

    res = run_bass_kernel_spmd(nc, in_maps, core_ids=list(range(NCORES)))
    total = sum(float(res.results[c]["outp"][0, 0]) for c in range(NCORES))
    return np.asarray(np.float32(total / B))


if __name__ == "__main__":
    rng = np.random.default_rng(0)
    f = rng.standard_normal((B, D)).astype(np.float32)
    g = rng.standard_normal((B, D)).astype(np.float32)
    t = rng.integers(0, 1000, size=B).astype(np.int64)
    print("loss:", kernel(f, g, t))
